# revision 19
# baseline (speedup 1.0000x reference)
"""Windowed multi-head attention block (qkv -> local attention -> o_proj -> LN)
for Trainium2, data-parallel over batch across 8 NeuronCores.

Shapes (hardcoded): B=8, S=1024, D=1024, H=16, DH=64, WIN=128, fp32 I/O.

Per-core dataflow (core b handles batch element b; no collectives):
  xT      = transpose(x)                      (PE transposes, fp32)
  v       = xT_chunk.T @ W_v                  (token-major, cast to bf16)
  per head-pair hp (B+D interleaved so PE never starves):
    qT,kT = W_chunk.T @ xT                    (feature-major, fp32r, out fp16)
    per q-block of 128 (batched 4 at a time): 384-wide key strip
      scores  = qT_blk.T @ kT_strip           (fp16 matmul, fp32 PSUM)
      probs   = exp(scale*scores) * mask01    (ACT exp -> bf16, DVE mask)
      probs  /= rowsum(probs)                 (GpSimd reduce, DVE recip+mul)
      probsT  = XBAR transpose, 24 tiles/instr(bf16)
      outT    = v_strip.T @ probsT            (bf16, accumulated over 3 blocks)
  proj    = attn_outT_chunk.T @ W_o           (token-major, fp32r)
  y       = LayerNorm(x + proj) * gamma + beta
"""

from contextlib import ExitStack

import numpy as np
import ml_dtypes

import concourse.bass as bass
import concourse.tile as tile
from concourse import bacc, mybir
from concourse.bass_utils import run_bass_kernel_spmd
from concourse.masks import make_identity

B, S, D = 8, 1024, 1024
H, DH = 16, 64
WIN = 128
SCALE = 1.0 / DH**0.5
LN_EPS = 1e-5
N_CORES = 8
P = 128
NSB = S // P          # 8 query/row blocks
NDC = D // P          # 8 feature chunks
KW = 3 * P            # 384-wide key strip
F32 = mybir.dt.float32
F32R = mybir.dt.float32r
F16 = mybir.dt.float16
BF16 = mybir.dt.bfloat16

_CACHE = {}


def _koff(qb):
    if qb == 0:
        return 0
    if qb == NSB - 1:
        return S - KW
    return (qb - 1) * P


def _build_masks():
    """[128, 2, 384] 0/1 masks (bf16) for first / interior / last q-blocks,
    replicated along the head-pair dim so the mask multiply is contiguous."""
    i = np.arange(P)[:, None]
    j = np.arange(KW)[None, :]
    masks = []
    for qb in (0, 1, NSB - 1):
        koff = _koff(qb)
        m = (np.abs((qb * P + i) - (koff + j)) <= WIN).astype(np.float32)
        m2 = np.broadcast_to(m[:, None, :], (P, 2, KW)).copy()
        masks.append(m2.astype(ml_dtypes.bfloat16))
    return masks  # first, interior, last


def _emit(nc):
    x = nc.dram_tensor("x", [S, D], F32, kind="ExternalInput").ap()
    w_qkv = nc.dram_tensor("w_qkv", [D, 3 * H * DH], F32, kind="ExternalInput").ap()
    b_qkv = nc.dram_tensor("b_qkv", [3 * H * DH], F32, kind="ExternalInput").ap()
    w_o = nc.dram_tensor("w_o", [H * DH, D], F32, kind="ExternalInput").ap()
    gamma = nc.dram_tensor("gamma", [D], F32, kind="ExternalInput").ap()
    beta = nc.dram_tensor("beta", [D], F32, kind="ExternalInput").ap()
    mask_f = nc.dram_tensor("mask_f", [P, 2, KW], BF16, kind="ExternalInput").ap()
    mask_i = nc.dram_tensor("mask_i", [P, 2, KW], BF16, kind="ExternalInput").ap()
    mask_l = nc.dram_tensor("mask_l", [P, 2, KW], BF16, kind="ExternalInput").ap()
    y = nc.dram_tensor("y", [S, D], F32, kind="ExternalOutput").ap()

    with tile.TileContext(nc) as tc, ExitStack() as ctx:
        consts = ctx.enter_context(tc.tile_pool(name="consts", bufs=1))
        ident = consts.tile([P, P], F32, tag="ident")
        make_identity(nc, ident)
        m_f = consts.tile([P, 2, KW], BF16, tag="m_f")
        m_i = consts.tile([P, 2, KW], BF16, tag="m_i")
        m_l = consts.tile([P, 2, KW], BF16, tag="m_l")
        nc.scalar.dma_start(out=m_f, in_=mask_f)
        nc.scalar.dma_start(out=m_i, in_=mask_i)
        nc.scalar.dma_start(out=m_l, in_=mask_l)
        bqk_sb = consts.tile([P, 16], F32, tag="bqk")  # q,k bias per out-chunk
        nc.scalar.dma_start(
            out=bqk_sb, in_=b_qkv[0 : 2 * H * DH].rearrange("(o p) -> p o", p=P)
        )

        def _bcast_load(dst, src):
            # replicate a [N] DRAM vector across all 128 partitions
            rep = bass.AP(tensor=src.tensor, offset=src.offset, ap=[[0, P]] + src.ap)
            nc.gpsimd.dma_start(out=dst, in_=rep)

        bv_sb = consts.tile([P, H * DH], F32, tag="bv")
        _bcast_load(bv_sb, b_qkv[2 * H * DH :])
        gamma_sb = consts.tile([P, D], F32, tag="gamma")
        _bcast_load(gamma_sb, gamma)
        beta_sb = consts.tile([P, D], F32, tag="beta")
        _bcast_load(beta_sb, beta)
        eps_sb = consts.tile([P, 1], F32, tag="eps")
        nc.vector.memset(eps_sb, LN_EPS)

        # PSUM pools (8 banks total: 2 + 2*2 + 2)
        ps_big = ctx.enter_context(tc.tile_pool(name="ps_big", bufs=2, space="PSUM"))
        ps_s = ctx.enter_context(tc.tile_pool(name="ps_s", bufs=2, space="PSUM"))
        ps_128 = ctx.enter_context(tc.tile_pool(name="ps_128", bufs=2, space="PSUM"))

        # attn-out lives into phase E; opened before qkv pool (LIFO close order)
        aoT_pool = ctx.enter_context(tc.tile_pool(name="aoTp", bufs=1))
        aoT = aoT_pool.tile([P, H // 2, S], F32R, tag="aoT")

        qkv_ctx = ctx.enter_context(ExitStack())
        qkv_pool = qkv_ctx.enter_context(tc.tile_pool(name="qkv", bufs=1))
        q_sb = qkv_pool.tile([P, H // 2, S], F16, tag="q_sb")
        k_sb = qkv_pool.tile([P, H // 2, S], F16, tag="k_sb")
        v_sb = qkv_pool.tile([P, NSB, H, DH], BF16, tag="v_sb")

        with ExitStack() as abc_ctx:
            xT_pool = abc_ctx.enter_context(tc.tile_pool(name="xTp", bufs=1))
            w_pool = abc_ctx.enter_context(tc.tile_pool(name="w_qkv", bufs=2))
            pr_pool = abc_ctx.enter_context(tc.tile_pool(name="probs", bufs=2))
            sm_pool = abc_ctx.enter_context(tc.tile_pool(name="smalls", bufs=8))

            # ---------- Phase A: xT via PE transposes (x read from DRAM) ----
            xT = xT_pool.tile([P, NDC, S], F32R, tag="xT")
            with tc.tile_pool(name="xin", bufs=2) as xin_pool:
                for sb in range(NSB):
                    xin = xin_pool.tile([P, D], F32, tag="xin")
                    nc.scalar.dma_start(out=xin, in_=x[sb * P : (sb + 1) * P, :])
                    for dc in range(NDC):
                        pst = ps_128.tile([P, P], F32, tag="ps128")
                        nc.tensor.transpose(pst, xin[:, dc * P : (dc + 1) * P], ident)
                        nc.any.tensor_copy(
                            out=xT[:, dc, sb * P : (sb + 1) * P], in_=pst
                        )

            # ---------- Phase C: v token-major, bf16 ------------------------
            for vh in range(2):
                wv = w_pool.tile([P, NDC, 512], F32R, tag="w")
                nc.scalar.dma_start(
                    out=wv,
                    in_=w_qkv[:, 2048 + vh * 512 : 2048 + (vh + 1) * 512]
                    .rearrange("(o p) c -> p o c", p=P)
                    .bitcast(F32R),
                )
                for sb in range(NSB):
                    ps = ps_big.tile([P, 512], F32, tag="ps_big")
                    for dc in range(NDC):
                        nc.tensor.matmul(
                            ps,
                            lhsT=xT[:, dc, sb * P : (sb + 1) * P],
                            rhs=wv[:, dc, :],
                            start=(dc == 0),
                            stop=(dc == NDC - 1),
                        )
                    nc.vector.tensor_tensor(
                        out=v_sb[:, sb, vh * 8 : (vh + 1) * 8, :],
                        in0=ps.rearrange("p (h e) -> p h e", e=DH),
                        in1=bv_sb[:, vh * 512 : (vh + 1) * 512].rearrange(
                            "p (h e) -> p h e", e=DH
                        ),
                        op=mybir.AluOpType.add,
                    )

            # ---------- Phases B+D interleaved per head-pair ----------------
            for hp in range(H // 2):
                # B: q chunk (oc=hp) and k chunk (oc=8+hp), feature-major fp16
                for oc in (hp, 8 + hp):
                    wt = w_pool.tile([P, NDC, P], F32R, tag="w")
                    nc.scalar.dma_start(
                        out=wt,
                        in_=w_qkv[:, oc * P : (oc + 1) * P]
                        .rearrange("(o p) c -> p o c", p=P)
                        .bitcast(F32R),
                    )
                    dst = q_sb if oc < 8 else k_sb
                    for sh in range(2):
                        ps = ps_big.tile([P, 512], F32, tag="ps_big")
                        for dc in range(NDC):
                            nc.tensor.matmul(
                                ps,
                                lhsT=wt[:, dc, :],
                                rhs=xT[:, dc, sh * 512 : (sh + 1) * 512],
                                start=(dc == 0),
                                stop=(dc == NDC - 1),
                            )
                        nc.scalar.activation(
                            out=dst[:, hp, sh * 512 : (sh + 1) * 512],
                            in_=ps,
                            func=mybir.ActivationFunctionType.Identity,
                            bias=bqk_sb[:, oc : oc + 1],
                        )

                # D: windowed attention for this head pair, all 8 q-blocks,
                # grouped by 4 for the batched XBAR transpose
                for qg in range(2):
                    pn_h = pr_pool.tile([P, 8, KW], BF16, tag="pn")
                    for qbl in range(4):
                        qb = qg * 4 + qbl
                        koff = _koff(qb)
                        msk = m_f if qb == 0 else (m_l if qb == NSB - 1 else m_i)
                        pss = ps_s.tile([P, 2, 512], F32, tag="ps_s")
                        for hh in range(2):
                            pb = hh * 64
                            nc.tensor.matmul(
                                pss[:, hh, 0:KW],
                                lhsT=q_sb[pb : pb + 64, hp, qb * P : (qb + 1) * P],
                                rhs=k_sb[pb : pb + 64, hp, koff : koff + KW],
                                start=True,
                                stop=True,
                            )
                        pu = pr_pool.tile([P, 2, KW], BF16, tag="pu")
                        nc.scalar.activation(
                            out=pu,
                            in_=pss[:, :, 0:KW],
                            func=mybir.ActivationFunctionType.Exp,
                            scale=SCALE,
                        )
                        pm = pr_pool.tile([P, 2, KW], BF16, tag="pm")
                        nc.gpsimd.tensor_tensor(
                            out=pm, in0=pu, in1=msk, op=mybir.AluOpType.mult
                        )
                        sm = sm_pool.tile([P, 2], F32, tag="sm")
                        nc.vector.reduce_sum(sm, pm, axis=mybir.AxisListType.X)
                        rs = sm_pool.tile([P, 2], F32, tag="rs")
                        nc.vector.reciprocal(rs, sm)
                        for hh in range(2):
                            nc.vector.tensor_scalar_mul(
                                pn_h[:, 2 * qbl + hh, :],
                                pm[:, hh, :],
                                rs[:, hh : hh + 1],
                            )
                    pt_h = pr_pool.tile([P, 24, P], BF16, tag="pt")
                    nc.sync.dma_start_transpose(
                        pt_h, pn_h.rearrange("p a b -> p (a b)")
                    )
                    for qbl in range(4):
                        qb = qg * 4 + qbl
                        kb0 = _koff(qb) // P
                        ps_av = ps_128.tile([P, P], F32, tag="ps128")
                        for hh in range(2):
                            h = 2 * hp + hh
                            pb = hh * 64
                            for c in range(3):
                                nc.tensor.matmul(
                                    ps_av[pb : pb + 64, :],
                                    lhsT=v_sb[:, kb0 + c, h, :],
                                    rhs=pt_h[:, (2 * qbl + hh) * 3 + c, :],
                                    start=(c == 0),
                                    stop=(c == 2),
                                )
                        nc.scalar.activation(
                            out=aoT[:, hp, qb * P : (qb + 1) * P],
                            in_=ps_av,
                            func=mybir.ActivationFunctionType.Copy,
                        )

        qkv_ctx.close()  # free q/k/v before o_net weights arrive

        # ---------- Phase E: o_net + residual + LayerNorm ----------
        with ExitStack() as o_ctx:
            wo_pool = o_ctx.enter_context(tc.tile_pool(name="w_o", bufs=2))
            ln_pool = o_ctx.enter_context(tc.tile_pool(name="ln", bufs=2))
            lnsm_pool = o_ctx.enter_context(tc.tile_pool(name="ln_sm", bufs=4))

            wo_tiles = []
            for dh2 in range(2):
                wo = wo_pool.tile([P, NDC, 512], F32R, tag="w_o")
                nc.scalar.dma_start(
                    out=wo,
                    in_=w_o[:, dh2 * 512 : (dh2 + 1) * 512]
                    .rearrange("(o p) c -> p o c", p=P)
                    .bitcast(F32R),
                )
                wo_tiles.append(wo)

            for sb in range(NSB):
                xr = ln_pool.tile([P, D], F32, tag="xr")
                nc.scalar.dma_start(out=xr, in_=x[sb * P : (sb + 1) * P, :])
                y0 = ln_pool.tile([P, D], F32, tag="y0")
                for dh2 in range(2):
                    ps = ps_big.tile([P, 512], F32, tag="ps_big")
                    for hc in range(NDC):
                        nc.tensor.matmul(
                            ps,
                            lhsT=aoT[:, hc, sb * P : (sb + 1) * P],
                            rhs=wo_tiles[dh2][:, hc, :],
                            start=(hc == 0),
                            stop=(hc == NDC - 1),
                        )
                    nc.vector.tensor_tensor(
                        out=y0[:, dh2 * 512 : (dh2 + 1) * 512],
                        in0=ps,
                        in1=xr[:, dh2 * 512 : (dh2 + 1) * 512],
                        op=mybir.AluOpType.add,
                    )
                stats = lnsm_pool.tile([P, 2, 6], F32, tag="stats")
                for g in range(2):
                    nc.vector.bn_stats(
                        out=stats[:, g, :], in_=y0[:, g * 512 : (g + 1) * 512]
                    )
                mv = lnsm_pool.tile([P, 2], F32, tag="mv")
                nc.vector.bn_aggr(out=mv, in_=stats)
                std = lnsm_pool.tile([P, 1], F32, tag="std")
                nc.scalar.activation(
                    out=std,
                    in_=mv[:, 1:2],
                    func=mybir.ActivationFunctionType.Sqrt,
                    bias=eps_sb,
                )
                rstd = lnsm_pool.tile([P, 1], F32, tag="rstd")
                nc.vector.reciprocal(rstd, std)
                # (y0 - mu) * rstd == y0 * rstd + (-mu * rstd), on ACT
                nmr = lnsm_pool.tile([P, 1], F32, tag="nmr")
                nc.vector.tensor_scalar(
                    out=nmr,
                    in0=mv[:, 0:1],
                    scalar1=-1.0,
                    scalar2=rstd,
                    op0=mybir.AluOpType.mult,
                    op1=mybir.AluOpType.mult,
                )
                y1 = ln_pool.tile([P, D], F32, tag="y1")
                nc.scalar.activation(
                    out=y1,
                    in_=y0,
                    func=mybir.ActivationFunctionType.Identity,
                    bias=nmr,
                    scale=rstd,
                )
                y2 = ln_pool.tile([P, D], F32, tag="y2")
                nc.gpsimd.tensor_tensor(
                    out=y2, in0=y1, in1=gamma_sb, op=mybir.AluOpType.mult
                )
                y3 = ln_pool.tile([P, D], F32, tag="y3")
                nc.gpsimd.tensor_tensor(
                    out=y3, in0=y2, in1=beta_sb, op=mybir.AluOpType.add
                )
                nc.scalar.dma_start(out=y[sb * P : (sb + 1) * P, :], in_=y3)
    return nc


def _get_compiled():
    if "nc" not in _CACHE:
        nc = bacc.Bacc("TRN2", target_bir_lowering=False, debug=False)
        _emit(nc)
        nc.compile()
        _CACHE["nc"] = nc
    return _CACHE["nc"]


def kernel(inp, W_qkv, b_qkv, W_o, gamma, beta):
    inp = np.ascontiguousarray(np.asarray(inp, dtype=np.float32))
    W_qkv = np.ascontiguousarray(np.asarray(W_qkv, dtype=np.float32))
    b_qkv = np.ascontiguousarray(np.asarray(b_qkv, dtype=np.float32))
    W_o = np.ascontiguousarray(np.asarray(W_o, dtype=np.float32))
    gamma = np.ascontiguousarray(np.asarray(gamma, dtype=np.float32))
    beta = np.ascontiguousarray(np.asarray(beta, dtype=np.float32))
    m_f, m_i, m_l = _build_masks()

    nc = _get_compiled()
    shared = {
        "w_qkv": W_qkv,
        "b_qkv": b_qkv,
        "w_o": W_o,
        "gamma": gamma,
        "beta": beta,
        "mask_f": m_f,
        "mask_i": m_i,
        "mask_l": m_l,
    }
    in_maps = [dict(shared, x=np.ascontiguousarray(inp[b])) for b in range(B)]
    res = run_bass_kernel_spmd(nc, in_maps, core_ids=list(range(N_CORES)))
    out = np.stack([res.results[b]["y"] for b in range(B)], axis=0)
    _CACHE["last_results"] = res
    return out
